# revision 7
# baseline (speedup 1.0000x reference)
"""BiLinearInteraction Trainium2 kernel (8 NeuronCores, data-parallel over batch).

Reference computation (per pair p=(i,j) of F=26 fields, P=325 pairs):
    out[b, p*64:(p+1)*64] = (x[i, b, :] @ W[p]) * x[j, b, :]
Full shapes: x [26, 4096, 64] f32, W [325, 64, 64] f32 -> out [4096, 20800] f32.

Strategy (v2)
- Shard batch 4096 -> 8 x 512, replicate W. All device data bf16; host
  upcasts the bf16 output to f32 (rel err ~3.5e-3, tol 2e-2).
- Deduplicated reads (27.4MB/core total vs 31.7 in v1): w stored once,
  split by left-field parity: even fields' blocks live in SBUF partitions
  0-63, odd fields' in 64-127 (each matmul only reads its parity's rows,
  so no zero padding and no duplicate copy). Same for the lhsT tensor.
- Matmuls: K=64 row-tiled (tile_position row groups 0/64 via base
  partition). Issue order interleaves the two parities so consecutive
  matmuls target disjoint PE row groups and execute concurrently
  (~2 cols/cycle effective).
- PSUM ring: 4 x [128,1024] f32 tiles (2 banks each, 16KB exact) so the
  PE runs ~1 pair ahead of the drain/multiply consumers.
- Elementwise (the kernel's engine-work bottleneck, ~83k cols/partition):
  per-piece ops greedily load-balanced across three engines:
    ACT   scalar.copy  PSUM f32 -> cp bf16 (drain)
    DVE   tensor_mul   cp x xn -> stage (2x bf16 mode) or fused
                       PSUM x xn -> stage (1x)
    POOL  tensor_mul   cp x xn -> stage (big pieces only; no PSUM port)
- DMA: input loads 6.1MB front-loaded (first-needed pieces on the ACT
  HWDGE ring, rest on the SP ring ahead of the writes); output writes
  21.3MB/core in ~0.5-0.8MB chunks on the SP ring.
"""

import sys

sys.path.insert(0, "/opt/trn_rl_repo")

from itertools import combinations

import ml_dtypes
import numpy as np

import concourse.bass as bass
import concourse.mybir as mybir
from concourse import bacc
from concourse.tile import TileContext

F, D, B = 26, 64, 4096
NCORES = 8
BC = B // NCORES          # 512 batch rows per core
NT = BC // 128            # 4 batch tiles of 128 rows
PAIRS = list(combinations(range(F), 2))
P = len(PAIRS)            # 325
OUT_COLS = P * D          # 20800

N_PAIRS = [F - 1 - i for i in range(F)]              # pairs with left field i
P_START = [sum(N_PAIRS[:i]) for i in range(F)]       # first pair index of field i

# Even/odd split of left fields (parity = PE row group).
EVEN_FIELDS = list(range(0, 25, 2))                  # 13 fields, 169 pairs
ODD_FIELDS = list(range(1, 25, 2))                   # 12 fields, 156 pairs
WE_COLS = sum(N_PAIRS[i] for i in EVEN_FIELDS) * D   # 10816
WO_COLS = sum(N_PAIRS[i] for i in ODD_FIELDS) * D    # 9984

# Per-field column offset within its parity's packed W tensor.
_off = 0
W_OFF = {}
for _i in EVEN_FIELDS:
    W_OFF[_i] = _off
    _off += N_PAIRS[_i] * D
_off = 0
for _i in ODD_FIELDS:
    W_OFF[_i] = _off
    _off += N_PAIRS[_i] * D

# W load groups (separate SBUF tiles => fine-grained DMA dependencies, the
# first pair's weights land ~2us in while the tail streams behind).
W_GROUPS = [[0], [1, 2], [3, 4, 5], list(range(6, 13))]   # by pair index k
WG_OF_PAIR = {}
for _gi, _g in enumerate(W_GROUPS):
    for _k in _g:
        WG_OF_PAIR[_k] = _gi

# Output/staging chunks in units of field pairs k (fields 2k, 2k+1).
CHUNKS = [[0], [1], [2], [3], [4], [5], [6, 7], [8, 9, 10, 11, 12]]

PIECE = 1024              # PSUM piece size (2 banks)
MMCOL = 512               # max matmul moving cols (1 bank)

F32 = mybir.dt.float32
BF16 = mybir.dt.bfloat16

# Greedy engine-balancing cost model (ns, HW-calibrated); rates per output
# column per 128-partition piece.
ACT_RATE, ACT_OV = 0.833, 260.0
DVE_R1, DVE_R2, DVE_OV = 1.042, 0.521, 150.0
POOL_RATE, POOL_OV, POOL_MIN = 1.98, 200.0, 384


def _field_pieces(cols):
    """Split a field's columns into <=PIECE chunks, multiples of 64."""
    n = -(-cols // PIECE)
    units = cols // D
    base, rem = divmod(units, n)
    sizes = [(base + (1 if j < rem else 0)) * D for j in range(n)]
    return sizes


def build_bass() -> bass.Bass:
    nc = bacc.Bacc()
    xn = nc.declare_dram_parameter("xn", [BC, F * D], BF16, isOutput=False)
    xt = nc.declare_dram_parameter("xt", [2 * D, NT * 13 * 128], BF16, isOutput=False)
    we = nc.declare_dram_parameter("we", [D, WE_COLS], BF16, isOutput=False)
    wo = nc.declare_dram_parameter("wo", [D, WO_COLS], BF16, isOutput=False)
    out = nc.declare_dram_parameter("out", [BC, OUT_COLS], BF16, isOutput=True)

    # Engine load tracker for the greedy consumer assignment (POOL starts
    # with the 4 early input-load DMA issues on the gpsimd queue).
    load = {"ACT": 0.0, "DVE": 1300.0, "POOL": 4 * 650.0}

    with TileContext(nc) as tc:
        with (
            tc.tile_pool(name="consts", bufs=1) as consts,
            tc.tile_pool(name="stage", bufs=6) as stage_pool,
            tc.tile_pool(name="psum", bufs=4, space="PSUM") as psum_pool,
        ):
            # --- W tiles: one [128, width] tile per load group; even parity
            # in rows 0-63, odd in rows 64-127. Within a group, each parity's
            # fields pack contiguously (group-local offsets).
            wg_tiles = []
            wg_off = []          # per group: {field: col offset in tile}
            wg_spans = []        # per group: (e0, e1, o0, o1) in we/wo cols
            for g in W_GROUPS:
                offs, ew, ow = {}, 0, 0
                for k in g:
                    i0, i1 = 2 * k, 2 * k + 1
                    offs[i0] = ew
                    ew += N_PAIRS[i0] * D
                    if i1 < 25:
                        offs[i1] = ow
                        ow += N_PAIRS[i1] * D
                e0 = W_OFF[2 * g[0]]
                o0 = W_OFF[2 * g[0] + 1] if 2 * g[0] + 1 < 25 else WO_COLS
                wg_spans.append((e0, e0 + ew, o0, o0 + ow))
                wg_off.append(offs)
                wg_tiles.append(
                    consts.tile([2 * D, max(ew, ow)], BF16, tag=f"wg{len(wg_tiles)}",
                                name=f"wg{len(wg_tiles)}")
                )

            xt_sb = [consts.tile([2 * D, 13 * 128], BF16, tag=f"xt{t}", name=f"xt{t}")
                     for t in range(NT)]
            xn_sb = [consts.tile([128, F * D], BF16, tag=f"xn{t}", name=f"xn{t}")
                     for t in range(NT)]

            # --- Input loads. First-needed pieces ride SWDGE (gpsimd), the
            # rest go on the SP HWDGE ring ahead of all output writes.
            e0, e1, o0, o1 = wg_spans[0]
            nc.gpsimd.dma_start(out=wg_tiles[0][0:D, : e1 - e0], in_=we[:, e0:e1])
            nc.gpsimd.dma_start(out=wg_tiles[0][D:2 * D, : o1 - o0], in_=wo[:, o0:o1])
            nc.gpsimd.dma_start(out=xt_sb[0][:], in_=xt[:, 0:13 * 128])
            nc.gpsimd.dma_start(out=xn_sb[0][:], in_=xn[0:128, :])
            for gi in (1, 2, 3):
                e0, e1, o0, o1 = wg_spans[gi]
                nc.sync.dma_start(out=wg_tiles[gi][0:D, : e1 - e0], in_=we[:, e0:e1])
                nc.sync.dma_start(out=wg_tiles[gi][D:2 * D, : o1 - o0], in_=wo[:, o0:o1])
            for t in range(1, NT):
                nc.sync.dma_start(out=xt_sb[t][:],
                                  in_=xt[:, t * 13 * 128:(t + 1) * 13 * 128])
                nc.sync.dma_start(out=xn_sb[t][:], in_=xn[t * 128:(t + 1) * 128, :])

            def consume(ps, cols, i, piece_off, st, st_off, xn_t, allow_pool=True):
                """Emit drain+mul (or fused) for one psum piece of field i.

                Drain writes straight into the staging tile (bf16); the
                multiply then runs in place (stage = stage * xn), so there is
                no intermediate cp ring to stall on.
                """
                xns = xn_t[:, (i + 1) * D + piece_off:(i + 1) * D + piece_off + cols]
                sts = st[:, st_off:st_off + cols]
                # Option B: fused on DVE (PSUM read, 1x)
                fused_dve = load["DVE"] + DVE_R1 * cols + DVE_OV
                cost_b = max(fused_dve, load["ACT"], load["POOL"])
                # Option A: ACT drain + in-place mul on DVE (2x) or POOL
                act_t = load["ACT"] + ACT_RATE * cols + ACT_OV
                mul_dve = load["DVE"] + DVE_R2 * cols + DVE_OV
                mul_pool = (load["POOL"] + POOL_RATE * cols + POOL_OV
                            if (allow_pool and cols >= POOL_MIN) else float("inf"))
                if max(act_t, mul_pool, load["DVE"]) < max(act_t, mul_dve, load["POOL"]):
                    pool_mul = True
                    cost_a = max(act_t, mul_pool, load["DVE"])
                else:
                    pool_mul = False
                    cost_a = max(act_t, mul_dve, load["POOL"])
                if cost_b <= cost_a:
                    load["DVE"] = fused_dve
                    nc.vector.tensor_mul(sts, ps[:, :cols], xns)
                else:
                    load["ACT"] = act_t
                    nc.scalar.copy(out=sts, in_=ps[:, :cols])
                    if pool_mul:
                        load["POOL"] = mul_pool
                        nc.gpsimd.tensor_mul(sts, sts, xns)
                    else:
                        load["DVE"] = mul_dve
                        nc.vector.tensor_mul(sts, sts, xns)

            for t in range(NT):
                xn_t = xn_sb[t]
                for chunk in CHUNKS:
                    c_p0 = P_START[2 * chunk[0]]
                    p_end = P_START[2 * chunk[-1] + 2] if 2 * chunk[-1] + 2 < 26 else P
                    c_cols = (p_end - c_p0) * D
                    st = stage_pool.tile([128, c_cols], BF16, tag="stage")
                    for k in chunk:
                        gi = WG_OF_PAIR[k]
                        wt = wg_tiles[gi]
                        offs = wg_off[gi]
                        # Build per-field piece lists, then interleave the
                        # two parities at piece granularity.
                        plists = []
                        for i in (2 * k, 2 * k + 1):
                            if i >= 25:
                                continue
                            cols_i = N_PAIRS[i] * D
                            sizes = _field_pieces(cols_i)
                            po = 0
                            lst = []
                            for sz in sizes:
                                lst.append((i, po, sz))
                                po += sz
                            plists.append(lst)
                        order = []
                        mx = max(len(l) for l in plists)
                        for j in range(mx):
                            for lst in plists:
                                if j < len(lst):
                                    order.append(lst[j])
                        # Process pieces two at a time, interleaving their
                        # matmul chunks: consecutive pieces alternate field
                        # parity, so adjacent matmuls hit disjoint PE row
                        # groups and execute concurrently (~2 cols/cycle).
                        last_chunk = (t == NT - 1) and (chunk is CHUNKS[-1])
                        for g0 in range(0, len(order), 2):
                            group = order[g0:g0 + 2]
                            tiles = []
                            mms = []   # per piece: list of (ps, lhsT, rhs_slice, s, n)
                            for (i, po, sz) in group:
                                r0 = (i % 2) * D
                                lhsT = xt_sb[t][r0:r0 + D,
                                                (i // 2) * 128:(i // 2 + 1) * 128]
                                ps = psum_pool.tile([128, PIECE], F32, tag="ps")
                                tiles.append(ps)
                                w0 = offs[i] + po
                                lst = []
                                for s in range(0, sz, MMCOL):
                                    n = min(MMCOL, sz - s)
                                    lst.append((ps, lhsT,
                                                wt[r0:r0 + D, w0 + s:w0 + s + n],
                                                s, n))
                                mms.append(lst)
                            mxm = max(len(l) for l in mms)
                            for j in range(mxm):
                                for lst in mms:
                                    if j < len(lst):
                                        ps, lhsT, rhs, s, n = lst[j]
                                        nc.tensor.matmul(
                                            ps[:, s:s + n], lhsT, rhs,
                                            start=True, stop=True,
                                        )
                            for (i, po, sz), ps in zip(group, tiles):
                                st_off = (P_START[i] - c_p0) * D + po
                                consume(ps, sz, i, po, st, st_off, xn_t,
                                        allow_pool=not last_chunk)
                    nc.sync.dma_start(
                        out=out[t * 128:(t + 1) * 128, c_p0 * D:c_p0 * D + c_cols],
                        in_=st[:],
                    )
    nc.compile()
    return nc


def prep_inputs(x: np.ndarray, W: np.ndarray):
    """Full inputs -> per-core in_maps with pre-packed bf16 layouts."""
    x = np.ascontiguousarray(np.asarray(x, dtype=np.float32))
    W = np.ascontiguousarray(np.asarray(W, dtype=np.float32))
    # we/wo: [64, cols], col = (packed pair)*64 + e, grouped by left-field
    # parity in combinations order.
    we = np.concatenate(
        [W[P_START[i]:P_START[i] + N_PAIRS[i]].transpose(1, 0, 2).reshape(D, -1)
         for i in EVEN_FIELDS], axis=1).astype(ml_dtypes.bfloat16)
    wo = np.concatenate(
        [W[P_START[i]:P_START[i] + N_PAIRS[i]].transpose(1, 0, 2).reshape(D, -1)
         for i in ODD_FIELDS], axis=1).astype(ml_dtypes.bfloat16)
    we = np.ascontiguousarray(we)
    wo = np.ascontiguousarray(wo)
    in_maps = []
    for c in range(NCORES):
        xc = x[:, c * BC:(c + 1) * BC, :]                      # [26, 512, 64]
        xn = np.ascontiguousarray(
            xc.transpose(1, 0, 2).reshape(BC, F * D).astype(ml_dtypes.bfloat16)
        )
        # xt: [128, NT*13*128]; rows 0-63 = even fields (block k = field 2k),
        # rows 64-127 = odd fields (block k = field 2k+1; k=12 holds field 25,
        # never read). Cols: t-major, then field-block, then batch row.
        xe = (xc[0::2].reshape(13, NT, 128, D).transpose(3, 1, 0, 2)
              .reshape(D, NT * 13 * 128))
        xo = (xc[1::2].reshape(13, NT, 128, D).transpose(3, 1, 0, 2)
              .reshape(D, NT * 13 * 128))
        xt = np.ascontiguousarray(
            np.concatenate([xe, xo], axis=0).astype(ml_dtypes.bfloat16)
        )
        in_maps.append({"xn": xn, "xt": xt, "we": we, "wo": wo})
    return in_maps


_CACHED_NC = None


def kernel(x: np.ndarray, W: np.ndarray) -> np.ndarray:
    global _CACHED_NC
    from concourse.bass_utils import run_bass_kernel_spmd

    if _CACHED_NC is None:
        _CACHED_NC = build_bass()
    in_maps = prep_inputs(x, W)
    res = run_bass_kernel_spmd(_CACHED_NC, in_maps, list(range(NCORES)))
    shards = [
        np.asarray(res.results[c]["out"]).astype(np.float32) for c in range(NCORES)
    ]
    return np.concatenate(shards, axis=0)
